# revision 1
# baseline (speedup 1.0000x reference)
"""Trainium2 Bass kernel for CostVolumePrompt (masked-softmax cost volume).

Computation per (b, h):
  vol[i, j] = dot(lfeat[b,:,h,i], rfeat[b,:,h,j]) / sqrt(C)      (W x W)
  prob      = softmax(vol, axis=j) * tril(W, W)                  (mask AFTER softmax)
  corresp_i = sum_j prob[i,j] * j
  conf_i    = max_j prob[i,j]
  disp_i    = max((i - corresp_i)/W, 0.1)       (corresp_i <= i, so abs == i-corresp)
  out       = [fx*baseline/lfar / disp, conf]

Device strategy (8 cores, data-parallel over H):
  Each core owns H/8 = 16 rows for all 4 batches -> 64 (b,h) pairs.
  Per pair, 4 row-tiles of (128 i x 512 j):
    PE  : vol tile via fp32r matmul (lhsT = lfeat chunk, rhs = rfeat row)
    ACT : e = exp(vol/sqrt(C)) with accum_out -> denominator (free)
    DVE : tensor_tensor_reduce(e * widx)      -> masked weighted sum s1
          tensor_mask_reduce(max over masked e) -> numerator of conf
  Tiny per-row finals are batched per b on (128, 64) tiles.
"""

import math
import numpy as np
from contextlib import ExitStack

import concourse.bass as bass
import concourse.bacc as bacc
import concourse.tile as tile
from concourse import mybir
from concourse._compat import with_exitstack
from concourse.bass_utils import run_bass_kernel_spmd
from concourse.dve_ops import TENSOR_TENSOR_REDUCE, TENSOR_MASK_REDUCE

B, V, C, H, W = 4, 2, 128, 128, 512
NCORES = 8
HLOC = H // NCORES          # 16 h-rows per core
HGRP = 8                    # h-rows per DMA group
NT = HLOC * 4               # finals columns per batch (h*4 + mi) = 64
SCALE = 1.0 / math.sqrt(C)  # 1/sqrt(C) / TEMPERATURE
MIN_DISP = 0.1

F32 = mybir.dt.float32
F32R = mybir.dt.float32r
F16 = mybir.dt.float16


@with_exitstack
def _body(ctx: ExitStack, tc: "tile.TileContext", io: dict):
    nc = tc.nc
    lfeat, rfeat = io["lfeat"], io["rfeat"]
    widx, maskend, ivec, scales = io["widx"], io["maskend"], io["ivec"], io["scales"]
    out_dc, out_cf = io["out_dc"], io["out_cf"]

    singles = ctx.enter_context(tc.tile_pool(name="singles", bufs=1))
    feats = ctx.enter_context(tc.tile_pool(name="feats", bufs=6))
    psum = ctx.enter_context(tc.tile_pool(name="psum", bufs=5, space="PSUM"))
    psacc = ctx.enter_context(tc.tile_pool(name="psacc", bufs=2, space="PSUM"))
    epool = ctx.enter_context(tc.tile_pool(name="epool", bufs=6))
    scr = ctx.enter_context(tc.tile_pool(name="scr", bufs=3))
    acc = ctx.enter_context(tc.tile_pool(name="acc", bufs=2))
    fin = ctx.enter_context(tc.tile_pool(name="fin", bufs=2))

    widx_sb = singles.tile([128, 4, W], F32)
    nc.sync.dma_start(out=widx_sb[:], in_=widx[:, :, :])
    maskend_sb = singles.tile([128, 4], F32)
    nc.sync.dma_start(out=maskend_sb[:], in_=maskend[:, :])
    ivec_sb = singles.tile([128, NT], F32)
    nc.sync.dma_start(out=ivec_sb[:], in_=ivec[:, :])
    sc_sb = singles.tile([128, B], F32)
    nc.gpsimd.dma_start(out=sc_sb[:], in_=scales[:, :].to_broadcast((128, B)))

    for b in range(B):
        s1c = acc.tile([128, NT], F32, tag="s1c")
        dnc = psacc.tile([128, NT], F32, tag="dnc")
        c1c = acc.tile([128, NT], F32, tag="c1c")
        for h in range(HLOC):
            lf = feats.tile([128, W], F16, tag="lf")
            rf = feats.tile([128, W], F16, tag="rf")
            if b == 0 and h < 2:
                # HWDGE fp32 load + ScalarE cast: skips the slow SWDGE
                # cast-DMA warmup on the critical first tiles.
                lf32 = feats.tile([128, W], F32, tag="lf32")
                rf32 = feats.tile([128, W], F32, tag="rf32")
                nc.sync.dma_start(out=lf32[:], in_=lfeat[b, :, h, :])
                nc.sync.dma_start(out=rf32[:], in_=rfeat[b, :, h, :])
                nc.scalar.copy(out=lf[:], in_=lf32[:])
                nc.scalar.copy(out=rf[:], in_=rf32[:])
            else:
                nc.gpsimd.dma_start(out=lf[:], in_=lfeat[b, :, h, :])
                nc.gpsimd.dma_start(out=rf[:], in_=rfeat[b, :, h, :])
            if True:
                for mi in range(4):
                    t = h * 4 + mi
                    ext = (mi + 1) * 128
                    vol = psum.tile([128, W], F32, tag="vol")
                    nc.tensor.matmul(
                        vol[:],
                        lf[:, mi * 128:(mi + 1) * 128],
                        rf[:, :],
                        start=True,
                        stop=True,
                    )
                    e = epool.tile([128, W], F32, tag="e")
                    nc.scalar.activation(
                        out=e[:],
                        in_=vol[:],
                        func=mybir.ActivationFunctionType.Exp,
                        scale=SCALE,
                        accum_out=dnc[:, t:t + 1],
                    )
                    so = scr.tile([128, W], F32, tag="so")
                    nc.vector._custom_dve(
                        TENSOR_TENSOR_REDUCE,
                        out=so[:, :ext],
                        in0=e[:, :ext],
                        in1=widx_sb[:, mi, :ext],
                        s0=0.0,      # accum seed
                        s1=1.0,      # scale
                        accum_out=s1c[:, t:t + 1],
                    )
                    mo = scr.tile([128, W], F32, tag="mo")
                    nc.vector._custom_dve(
                        TENSOR_MASK_REDUCE,
                        out=mo[:, :ext],
                        in0=e[:, :ext],
                        in1=maskend_sb[:, mi:mi + 1],   # mask_end (C3 spill)
                        s0=0.0,      # mask_start
                        s1=0.0,      # accum seed (e > 0 always)
                        imm2=1.0,    # scale
                        accum_out=c1c[:, t:t + 1],
                    )
        # ---- batched finals for this b ----
        r = fin.tile([128, NT], F32, tag="r")
        nc.vector.reciprocal_approx_fast(out=r[:], in_=dnc[:])
        cf = fin.tile([128, NT], F32, tag="cf")
        nc.gpsimd.tensor_mul(out=cf[:], in0=c1c[:], in1=r[:])
        cor = fin.tile([128, NT], F32, tag="cor")
        nc.gpsimd.tensor_mul(out=cor[:], in0=s1c[:], in1=r[:])
        dd = fin.tile([128, NT], F32, tag="dd")
        nc.gpsimd.tensor_sub(out=dd[:], in0=ivec_sb[:], in1=cor[:])
        dcl = fin.tile([128, NT], F32, tag="dcl")
        nc.gpsimd.tensor_scalar(
            out=dcl[:], in0=dd[:], scalar1=1.0 / W, scalar2=MIN_DISP,
            op0=mybir.AluOpType.mult, op1=mybir.AluOpType.max,
        )
        r2 = fin.tile([128, NT], F32, tag="r2")
        nc.vector.reciprocal_approx_fast(out=r2[:], in_=dcl[:])
        od = fin.tile([128, NT], F32, tag="od")
        nc.vector.tensor_scalar(
            out=od[:], in0=r2[:], scalar1=sc_sb[:, b:b + 1], scalar2=None,
            op0=mybir.AluOpType.mult,
        )
        nc.sync.dma_start(out=out_dc[b, :, :], in_=od[:])
        nc.sync.dma_start(out=out_cf[b, :, :], in_=cf[:])


_NC_CACHE = None


def _build_nc():
    global _NC_CACHE
    if _NC_CACHE is not None:
        return _NC_CACHE
    nc = bacc.Bacc("TRN2", target_bir_lowering=False, debug=False)
    io = {
        "lfeat": nc.dram_tensor("lfeat", (B, C, HLOC, W), F32, kind="ExternalInput"),
        "rfeat": nc.dram_tensor("rfeat", (B, C, HLOC, W), F32, kind="ExternalInput"),
        "widx": nc.dram_tensor("widx", (128, 4, W), F32, kind="ExternalInput"),
        "maskend": nc.dram_tensor("maskend", (128, 4), F32, kind="ExternalInput"),
        "ivec": nc.dram_tensor("ivec", (128, NT), F32, kind="ExternalInput"),
        "scales": nc.dram_tensor("scales", (1, B), F32, kind="ExternalInput"),
        "out_dc": nc.dram_tensor("out_dc", (B, 128, NT), F32, kind="ExternalOutput"),
        "out_cf": nc.dram_tensor("out_cf", (B, 128, NT), F32, kind="ExternalOutput"),
    }
    with tile.TileContext(nc) as tc:
        _body(tc, io)
    nc.compile()
    _NC_CACHE = nc
    return nc


def _host_constants():
    p = np.arange(128)[:, None, None]
    mi = np.arange(4)[None, :, None]
    j = np.arange(W)[None, None, :]
    widx = np.where(j <= mi * 128 + p, j, 0).astype(np.float32)       # (128,4,W)
    maskend = (np.arange(4)[None, :] * 128 + np.arange(128)[:, None] + 1).astype(
        np.float32)                                                    # (128,4)
    iv = (np.arange(4)[None, :] * 128 + np.arange(128)[:, None]).astype(np.float32)
    ivec = np.tile(iv, (1, HLOC))                                      # (128, 64)
    return widx, maskend, ivec


def kernel(feat, extri, intri, near, far, _run_kwargs=None, _core_ids=None):
    feat = np.asarray(feat, dtype=np.float32)
    extri = np.asarray(extri, dtype=np.float32)
    intri = np.asarray(intri, dtype=np.float32)
    far = np.asarray(far, dtype=np.float32)

    fx = intri[:, 0, 0, 0]                                             # (B,)
    baseline = np.linalg.norm(extri[:, 0, :3, 3] - extri[:, 1, :3, 3], axis=-1)
    lfar = far[:, 0]
    scales = (fx * baseline / lfar).astype(np.float32).reshape(1, B)

    widx, maskend, ivec = _host_constants()
    core_ids = list(range(NCORES)) if _core_ids is None else _core_ids

    in_maps = []
    for ci in range(len(core_ids)):
        hs = slice(ci * HLOC, (ci + 1) * HLOC)
        in_maps.append({
            "lfeat": np.ascontiguousarray(feat[:, 0, :, hs, :]),
            "rfeat": np.ascontiguousarray(feat[:, 1, :, hs, :]),
            "widx": widx, "maskend": maskend, "ivec": ivec, "scales": scales,
        })

    nc = _build_nc()
    res = run_bass_kernel_spmd(nc, in_maps, core_ids=core_ids,
                               **(_run_kwargs or {}))

    out = np.zeros((B, 1, 2, H, W), dtype=np.float32)
    for ci in range(len(core_ids)):
        h0 = ci * HLOC
        dc = res.results[ci]["out_dc"]          # (B, 128, 64), col = h*4+mi
        cf = res.results[ci]["out_cf"]
        dc = dc.reshape(B, 128, HLOC, 4).transpose(0, 2, 3, 1).reshape(B, HLOC, W)
        cf = cf.reshape(B, 128, HLOC, 4).transpose(0, 2, 3, 1).reshape(B, HLOC, W)
        out[:, 0, 0, h0:h0 + HLOC, :] = dc
        out[:, 0, 1, h0:h0 + HLOC, :] = cf
    if _run_kwargs:
        kernel.last_results = res
    return out



# revision 6
# speedup vs baseline: 1.0283x; 1.0283x over previous
"""Trainium2 Bass kernel for CostVolumePrompt (masked-softmax cost volume).

Computation per (b, h):
  vol[i, j] = dot(lfeat[b,:,h,i], rfeat[b,:,h,j]) / sqrt(C)      (W x W)
  prob      = softmax(vol, axis=j) * tril(W, W)                  (mask AFTER softmax)
  corresp_i = sum_j prob[i,j] * j
  conf_i    = max_j prob[i,j]
  disp_i    = max((i - corresp_i)/W, 0.1)       (corresp_i <= i, so abs == i-corresp)
  out       = [fx*baseline/lfar / disp, conf]

Device strategy (8 cores, data-parallel over H):
  Each core owns H/8 = 16 rows for all 4 batches -> 64 (b,h) pairs.
  Per pair, 4 row-tiles of (128 i x 512 j):
    PE  : vol tile via fp32r matmul (lhsT = lfeat chunk, rhs = rfeat row)
    ACT : e = exp(vol/sqrt(C)) with accum_out -> denominator (free)
    DVE : tensor_tensor_reduce(e * widx)      -> masked weighted sum s1
          tensor_mask_reduce(max over masked e) -> numerator of conf
  Tiny per-row finals are batched per b on (128, 64) tiles.
"""

import math
import numpy as np
from contextlib import ExitStack

import concourse.bass as bass
import concourse.bacc as bacc
import concourse.tile as tile
from concourse import mybir
from concourse._compat import with_exitstack
from concourse.bass_utils import run_bass_kernel_spmd
from concourse.dve_ops import TENSOR_TENSOR_REDUCE, TENSOR_MASK_REDUCE

B, V, C, H, W = 4, 2, 128, 128, 512
NCORES = 8
HLOC = H // NCORES          # 16 h-rows per core
HGRP = 8                    # h-rows per DMA group
NT = HLOC * 4               # finals columns per batch (h*4 + mi) = 64
SCALE = 1.0 / math.sqrt(C)  # 1/sqrt(C) / TEMPERATURE
MIN_DISP = 0.1

F32 = mybir.dt.float32
F32R = mybir.dt.float32r
F16 = mybir.dt.float16


@with_exitstack
def _body(ctx: ExitStack, tc: "tile.TileContext", io: dict):
    nc = tc.nc
    lfeat, rfeat = io["lfeat"], io["rfeat"]
    widx, maskend, ivec, scales = io["widx"], io["maskend"], io["ivec"], io["scales"]
    out_dc, out_cf = io["out_dc"], io["out_cf"]

    singles = ctx.enter_context(tc.tile_pool(name="singles", bufs=1))
    feats = ctx.enter_context(tc.tile_pool(name="feats", bufs=6))
    psum = ctx.enter_context(tc.tile_pool(name="psum", bufs=5, space="PSUM"))
    psacc = ctx.enter_context(tc.tile_pool(name="psacc", bufs=2, space="PSUM"))
    epool = ctx.enter_context(tc.tile_pool(name="epool", bufs=6))
    scr = ctx.enter_context(tc.tile_pool(name="scr", bufs=3))
    acc = ctx.enter_context(tc.tile_pool(name="acc", bufs=2))
    fin = ctx.enter_context(tc.tile_pool(name="fin", bufs=2))

    widx_sb = singles.tile([128, 4, W], F32)
    nc.sync.dma_start(out=widx_sb[:], in_=widx[:, :, :])
    maskend_sb = singles.tile([128, 4], F32)
    nc.sync.dma_start(out=maskend_sb[:], in_=maskend[:, :])
    ivec_sb = singles.tile([128, NT], F32)
    nc.sync.dma_start(out=ivec_sb[:], in_=ivec[:, :])
    sc_sb = singles.tile([128, B], F32)
    nc.gpsimd.dma_start(out=sc_sb[:], in_=scales[:, :].to_broadcast((128, B)))

    for b in range(B):
        s1c = acc.tile([128, NT], F32, tag="s1c")
        dnc = psacc.tile([128, NT], F32, tag="dnc")
        c1c = acc.tile([128, NT], F32, tag="c1c")
        for h in range(HLOC):
            lf = feats.tile([128, W], F16, tag="lf")
            rf = feats.tile([128, W], F16, tag="rf")
            nc.sync.dma_start(out=lf[:], in_=lfeat[b, :, h, :])
            nc.sync.dma_start(out=rf[:], in_=rfeat[b, :, h, :])
            if True:
                for mi in range(4):
                    t = h * 4 + mi
                    ext = (mi + 1) * 128
                    vol = psum.tile([128, W], F32, tag="vol")
                    nc.tensor.matmul(
                        vol[:],
                        lf[:, mi * 128:(mi + 1) * 128],
                        rf[:, :],
                        start=True,
                        stop=True,
                    )
                    e = epool.tile([128, W], F32, tag="e")
                    nc.scalar.activation(
                        out=e[:],
                        in_=vol[:],
                        func=mybir.ActivationFunctionType.Exp,
                        scale=SCALE,
                        accum_out=dnc[:, t:t + 1],
                    )
                    so = scr.tile([128, W], F32, tag="so")
                    nc.vector._custom_dve(
                        TENSOR_TENSOR_REDUCE,
                        out=so[:, :ext],
                        in0=e[:, :ext],
                        in1=widx_sb[:, mi, :ext],
                        s0=0.0,      # accum seed
                        s1=1.0,      # scale
                        accum_out=s1c[:, t:t + 1],
                    )
                    mo = scr.tile([128, W], F32, tag="mo")
                    nc.vector._custom_dve(
                        TENSOR_MASK_REDUCE,
                        out=mo[:, :ext],
                        in0=e[:, :ext],
                        in1=maskend_sb[:, mi:mi + 1],   # mask_end (C3 spill)
                        s0=0.0,      # mask_start
                        s1=0.0,      # accum seed (e > 0 always)
                        imm2=1.0,    # scale
                        accum_out=c1c[:, t:t + 1],
                    )
        # ---- batched finals for this b ----
        r = fin.tile([128, NT], F32, tag="r")
        nc.vector.reciprocal_approx_fast(out=r[:], in_=dnc[:])
        cf = fin.tile([128, NT], F32, tag="cf")
        nc.gpsimd.tensor_mul(out=cf[:], in0=c1c[:], in1=r[:])
        cor = fin.tile([128, NT], F32, tag="cor")
        nc.gpsimd.tensor_mul(out=cor[:], in0=s1c[:], in1=r[:])
        dd = fin.tile([128, NT], F32, tag="dd")
        nc.gpsimd.tensor_sub(out=dd[:], in0=ivec_sb[:], in1=cor[:])
        dcl = fin.tile([128, NT], F32, tag="dcl")
        nc.gpsimd.tensor_scalar(
            out=dcl[:], in0=dd[:], scalar1=1.0 / W, scalar2=MIN_DISP,
            op0=mybir.AluOpType.mult, op1=mybir.AluOpType.max,
        )
        r2 = fin.tile([128, NT], F32, tag="r2")
        nc.vector.reciprocal_approx_fast(out=r2[:], in_=dcl[:])
        od = fin.tile([128, NT], F32, tag="od")
        nc.vector.tensor_scalar(
            out=od[:], in0=r2[:], scalar1=sc_sb[:, b:b + 1], scalar2=None,
            op0=mybir.AluOpType.mult,
        )
        nc.sync.dma_start(out=out_dc[b, :, :], in_=od[:])
        nc.sync.dma_start(out=out_cf[b, :, :], in_=cf[:])


_NC_CACHE = None


def _build_nc():
    global _NC_CACHE
    if _NC_CACHE is not None:
        return _NC_CACHE
    nc = bacc.Bacc("TRN2", target_bir_lowering=False, debug=False)
    io = {
        "lfeat": nc.dram_tensor("lfeat", (B, C, HLOC, W), F16, kind="ExternalInput"),
        "rfeat": nc.dram_tensor("rfeat", (B, C, HLOC, W), F16, kind="ExternalInput"),
        "widx": nc.dram_tensor("widx", (128, 4, W), F32, kind="ExternalInput"),
        "maskend": nc.dram_tensor("maskend", (128, 4), F32, kind="ExternalInput"),
        "ivec": nc.dram_tensor("ivec", (128, NT), F32, kind="ExternalInput"),
        "scales": nc.dram_tensor("scales", (1, B), F32, kind="ExternalInput"),
        "out_dc": nc.dram_tensor("out_dc", (B, 128, NT), F32, kind="ExternalOutput"),
        "out_cf": nc.dram_tensor("out_cf", (B, 128, NT), F32, kind="ExternalOutput"),
    }
    with tile.TileContext(nc) as tc:
        _body(tc, io)
    nc.compile()
    _NC_CACHE = nc
    return nc


def _host_constants():
    p = np.arange(128)[:, None, None]
    mi = np.arange(4)[None, :, None]
    j = np.arange(W)[None, None, :]
    widx = np.where(j <= mi * 128 + p, j, 0).astype(np.float32)       # (128,4,W)
    maskend = (np.arange(4)[None, :] * 128 + np.arange(128)[:, None] + 1).astype(
        np.float32)                                                    # (128,4)
    iv = (np.arange(4)[None, :] * 128 + np.arange(128)[:, None]).astype(np.float32)
    ivec = np.tile(iv, (1, HLOC))                                      # (128, 64)
    return widx, maskend, ivec


def kernel(feat, extri, intri, near, far, _run_kwargs=None, _core_ids=None):
    feat = np.asarray(feat, dtype=np.float32)
    extri = np.asarray(extri, dtype=np.float32)
    intri = np.asarray(intri, dtype=np.float32)
    far = np.asarray(far, dtype=np.float32)

    fx = intri[:, 0, 0, 0]                                             # (B,)
    baseline = np.linalg.norm(extri[:, 0, :3, 3] - extri[:, 1, :3, 3], axis=-1)
    lfar = far[:, 0]
    scales = (fx * baseline / lfar).astype(np.float32).reshape(1, B)

    widx, maskend, ivec = _host_constants()
    core_ids = list(range(NCORES)) if _core_ids is None else _core_ids

    feat16 = feat.astype(np.float16)
    in_maps = []
    for ci in range(len(core_ids)):
        hs = slice(ci * HLOC, (ci + 1) * HLOC)
        in_maps.append({
            "lfeat": np.ascontiguousarray(feat16[:, 0, :, hs, :]),
            "rfeat": np.ascontiguousarray(feat16[:, 1, :, hs, :]),
            "widx": widx, "maskend": maskend, "ivec": ivec, "scales": scales,
        })

    nc = _build_nc()
    res = run_bass_kernel_spmd(nc, in_maps, core_ids=core_ids,
                               **(_run_kwargs or {}))

    out = np.zeros((B, 1, 2, H, W), dtype=np.float32)
    for ci in range(len(core_ids)):
        h0 = ci * HLOC
        dc = res.results[ci]["out_dc"]          # (B, 128, 64), col = h*4+mi
        cf = res.results[ci]["out_cf"]
        dc = dc.reshape(B, 128, HLOC, 4).transpose(0, 2, 3, 1).reshape(B, HLOC, W)
        cf = cf.reshape(B, 128, HLOC, 4).transpose(0, 2, 3, 1).reshape(B, HLOC, W)
        out[:, 0, 0, h0:h0 + HLOC, :] = dc
        out[:, 0, 1, h0:h0 + HLOC, :] = cf
    if _run_kwargs:
        kernel.last_results = res
    return out

